# revision 106
# baseline (speedup 1.0000x reference)
"""FaceAttnProcessor Trainium2 kernel (v4, fp8 DoubleRow).

Sharding: 8 cores = batch(2) x row-slices(4 x 256 rows). Each core computes
its 256 query rows end-to-end (self-attn with redundant K/V over the full
1040-token sequence, GEGLU FF, cross-attn against the 77 text tokens).
No collectives; the host scatters inputs and gathers the 8 row-slices.

Layout/schedule:
- All big GEMMs run as fp8e4 DoubleRow matmuls (0.5 PE cycles/row, two
  128-deep k-tiles folded per instruction): QKV/CA projections pair the
  KC dim, attnV pairs sequence chunks (est tiles hold rc pairs), the
  O-projs pair heads. Scores stay bf16 (64-deep contraction). Weights
  are host-packed fp8 x64 (x16 for w1) against subnormal flush; the
  unscales fold into existing copyback/gelu scales and host-side
  tanh(alpha)/scale factors (alph carries tanh(aa), tanh(ad)/1024,
  tanh(aa)/4096).
- LN stats via one-pass DVE BN_STATS (bf16 stats tile), software-
  pipelined one chunk ahead so cT copybacks never head-of-line block
  the next chunk's stats; copybacks are spread Act/DVE/Pool.
- Q^T/K^T production is streamed inside the SA loop (make_qkt(f) just
  ahead of scores(f), sharing the score-PSUM bufs) so the Act exp
  stream starts as soon as cT completes.
- SA softmax row-sums are free: V carries a ones-column, one f32r
  reciprocal per hp covers both heads, broadcast by a 1-row PE matmul.
- FF1 runs 4 accumulation chains per fc into one PSUM tile -> a single
  gelu + a single a*g multiply per fc; FF entry overlaps the qc1 O-proj
  with the rc0 LN. x2 and x2^T are produced per-f inside the FF2 loop
  (x2T feeds the CA Q DoubleRow matmul), so CA entry has no transpose
  tail. CA O-proj runs hq-outer across all four output blocks.
- Weight stream on the Pool/SWDGE queue in consumption order; the FF
  weight DMAs issue after the chunk loop so Pool stays free for V
  copybacks during it. Act tables (Sqrt/Identity/Exp/Gelu) pre-warmed
  off the critical path.
"""
import os
import numpy as np
from contextlib import ExitStack, nullcontext

import concourse.bass as bass
import concourse.tile as tile
import concourse.mybir as mybir
from concourse import bacc
from concourse.bass_utils import run_bass_kernel_spmd
from concourse.masks import make_identity

F32 = mybir.dt.float32
F32R = mybir.dt.float32r
BF16 = mybir.dt.bfloat16
F8 = mybir.dt.float8e4
DR = mybir.MatmulPerfMode.DoubleRow
AFT = mybir.ActivationFunctionType
W1S = 16.0                 # host scale on ff_w1 (fp8 subnormal headroom)
W2S = 64.0                 # host scale on ff_w2
WS = 64.0                  # host scale on qkv/ca projection weights
WSI = 1.0 / WS

P = 128
B, N, C, L = 2, 1024, 768, 93
NT, NF = 77, 16            # text / face tokens
NTP = 80                   # text tokens padded
NC_ = 1040                 # N + NF combined sequence
R = 256                    # query rows per core
H, D = 12, 64              # heads, head dim
HP = 6                     # head pairs
INNER = 3072
KC = 6                     # C // 128
EPS = 1e-5

_cache = {}


def build(fast_ln2=False):
    nc = bacc.Bacc("TRN2", target_bir_lowering=False, debug=False, num_devices=8)

    x_full_d = nc.dram_tensor("x_full", [N, C], F32, kind="ExternalInput")
    xb_d = nc.dram_tensor("xb", [P, 8, C], BF16, kind="ExternalInput")
    face_d = nc.dram_tensor("face", [NF, C], F32, kind="ExternalInput")
    ehsT_d = nc.dram_tensor("ehsT", [P, KC, NTP], F8, kind="ExternalInput")
    lnvT_d = nc.dram_tensor("lnvT", [P, KC, 4], F32, kind="ExternalInput")
    bcast_d = nc.dram_tensor("bcast", [P, 4, C], F32, kind="ExternalInput")
    alph_d = nc.dram_tensor("alph", [1, 3], F32, kind="ExternalInput")
    wv_d = nc.dram_tensor("wv", [P, KC, C], F8, kind="ExternalInput")
    wk_d = nc.dram_tensor("wk", [P, KC, C], F8, kind="ExternalInput")
    wq_d = nc.dram_tensor("wq", [P, KC, C], F8, kind="ExternalInput")
    ck_d = nc.dram_tensor("ck", [P, KC, C], F8, kind="ExternalInput")
    cv_d = nc.dram_tensor("cv", [P, KC, C], F8, kind="ExternalInput")
    wo_d = nc.dram_tensor("wo", [D, H, C], F8, kind="ExternalInput")
    w1_d = nc.dram_tensor("w1", [P, 12, KC, 2, 256], F8, kind="ExternalInput")
    w2_d = nc.dram_tensor("w2", [P, 24, C], F8, kind="ExternalInput")
    cq_d = nc.dram_tensor("cq", [P, KC, C], F8, kind="ExternalInput")
    co_d = nc.dram_tensor("co", [P, HP, C], F8, kind="ExternalInput")
    out_d = nc.dram_tensor("out_own", [R, C], F32, kind="ExternalOutput")

    with tile.TileContext(nc) as tc, ExitStack() as ctx:
        consts = ctx.enter_context(tc.tile_pool(name="consts", bufs=1))
        acts = ctx.enter_context(tc.tile_pool(name="acts", bufs=1))
        tmp1 = ctx.enter_context(tc.tile_pool(name="tmp1", bufs=1))
        tmp = ctx.enter_context(tc.tile_pool(name="tmp", bufs=2))
        dram = ctx.enter_context(tc.tile_pool(name="dram", bufs=1, space="DRAM"))

        # ---------------- input loads (SP queue): critical-path first ------
        xf = acts.tile([P, 8, C], BF16, tag="xf")
        nc.sync.dma_start(xf[:, 0, :], xb_d[:, 0, :])
        alo = consts.tile([1, 3], F32)
        nc.sync.dma_start(alo[:], alph_d[:])
        lnvT = consts.tile([P, KC, 4], F32, tag="lnvT")
        nc.sync.dma_start(lnvT[:], lnvT_d[:])
        for rc in range(1, 8):
            nc.sync.dma_start(xf[:, rc, :], xb_d[:, rc, :])
        face = consts.tile([NF, C], F32, tag="face")
        nc.sync.dma_start(face[:], face_d[:])
        ehsT = consts.tile([P, KC, NTP], F8, tag="ehsT")
        nc.sync.dma_start(ehsT[:], ehsT_d[:])

        # tanh(alpha) computed on host: alph = [tanh(aa), tanh(ad)/(W1S*W2S)]
        tA = consts.tile([P, 1], F32, tag="tA")
        nc.sync.dma_start(tA[:], alph_d[0:1, 0:1].to_broadcast([P, 1]))
        tD2 = consts.tile([P, 1], F32, tag="tD2")
        nc.sync.dma_start(tD2[:], alph_d[0:1, 1:2].to_broadcast([P, 1]))
        tA2 = consts.tile([P, 1], F32, tag="tA2")   # tanh(aa)/(OS*WS)
        nc.sync.dma_start(tA2[:], alph_d[0:1, 2:3].to_broadcast([P, 1]))
        obias = consts.tile([P, 2, C], F32, tag="obias")   # {sa_wo_b, ca_wo_b}
        nc.sync.dma_start(obias[:], bcast_d[:, 2:4, :])
        xo = acts.tile([P, 2, C], F32, tag="xo")
        nc.sync.dma_start(xo[:], x_full_d[0:R, :].rearrange(
            "(rc p) c -> p rc c", p=P))

        eps_t = consts.tile([P, 1], F32)
        nc.vector.memset(eps_t[:], EPS)
        actwarm = consts.tile([1, 6], F32)
        WARM = os.environ.get("KNOWARM", "") == ""
        TRUNC = int(os.environ.get("KTRUNC", "9"))

        def dbg_out(src_ap):
            do = acts.tile([P, 2, C], F32, tag="dbgout")
            nc.vector.tensor_copy(do[:, 0, :], src_ap)
            nc.vector.tensor_copy(do[:, 1, :], src_ap)
            for qc in range(2):
                nc.sync.dma_start(out_d[qc * P:(qc + 1) * P, :], do[:, qc, :])
        if WARM:
            nc.scalar.activation(actwarm[:, 0:1], alo[0:1, 0:1], AFT.Sqrt,
                                 scale=0.0, bias=eps_t[0:1, 0:1])
            # pre-pull the Identity table too: the first cT copyback would
            # otherwise load it mid-LN, stalling the first chunk's sqrt
            nc.scalar.activation(actwarm[:, 5:6], alo[0:1, 0:1], AFT.Identity,
                                 scale=eps_t[0:1, 0:1], bias=eps_t[0:1, 0:1])
        ones_r = consts.tile([1, P], F32R)
        nc.vector.memset(ones_r[:].bitcast(F32), 1.0)

        wobB, cobB = obias[:, 0, :], obias[:, 1, :]

        # ---------------- helpers ----------------
        def ln_stats(x_ap, p):
            """Normalized (x-m)/std of x_ap [p, 768], cast to bf16.
            mean/var via one-pass DVE BN_STATS (hw cap: 512 free/call)."""
            bnt = tmp.tile([P, 12], BF16, tag="ln_bnt")
            nc.vector.bn_stats(bnt[:p, 0:6], x_ap[:, 0:384])
            nc.vector.bn_stats(bnt[:p, 6:12], x_ap[:, 384:C])
            mv = tmp.tile([P, 2], F32, tag="ln_mv")
            nc.vector.bn_aggr(mv[:p], bnt[:p])
            std = tmp.tile([P, 1], F32, tag="ln_std")
            nc.scalar.activation(std[:p], mv[:p, 1:2], AFT.Sqrt,
                                 bias=eps_t[:p, 0:1])
            rstd = tmp.tile([P, 1], F32, tag="ln_rstd")
            nc.vector.reciprocal(rstd[:p], std[:p])
            xn = tmp.tile([P, C], BF16, tag="ln_xnb")
            nc.vector.tensor_scalar(xn[:p], x_ap, mv[:p, 0:1], rstd[:p],
                                    mybir.AluOpType.subtract, mybir.AluOpType.mult)
            return xn

        def transpose_gb(ps_t, xn, p, dst, col, gi, bi, flip=0, pool_mix=False):
            """PE-transpose bf16 xn [p,768] into dst[:, k, col:col+p] (bf16),
            applying per-channel gain lnvT[:,k,gi] / bias lnvT[:,k,bi].
            pool_mix: copybacks on Pool+Act (keeps DVE free, e.g. in FF1)."""
            for k in range(KC):
                pt = ps_t.tile([P, P], BF16, tag="tp")
                nc.tensor.transpose(pt[:, 0:p], xn[:p, bass.ts(k, P)],
                                    identB[:p, :p])
                if (k + flip) % 2 == 0:
                    nc.vector.tensor_scalar(
                        dst[:, k, col:col + p], pt[:, 0:p],
                        lnvT[:, k, gi:gi + 1], lnvT[:, k, bi:bi + 1],
                        mybir.AluOpType.mult, mybir.AluOpType.add)
                else:
                    nc.scalar.activation(
                        dst[:, k, col:col + p], pt[:, 0:p],
                        AFT.Identity, bias=lnvT[:, k, bi:bi + 1],
                        scale=lnvT[:, k, gi:gi + 1])

        # ---------------- persistent activations ----------------
        x1 = acts.tile([P, 2, C], F32, tag="x1")
        x2 = acts.tile([P, 2, C], F32, tag="x2")
        x2T = acts.tile([P, KC, R], F8, tag="x2T")
        cqt = acts.tile([P, KC, C], F8, tag="cqt")
        cot = acts.tile([P, HP, C], F8, tag="cot")
        KcaT = acts.tile([P, KC, NTP], BF16, tag="KcaT")
        Vca = acts.tile([NTP, H, D + 1], F8, tag="Vca")

        ctx2 = ExitStack()   # pools dead after FF; freed before CA
        saout = ctx2.enter_context(tc.tile_pool(name="saout", bufs=1))
        with nullcontext():
            attnU = saout.tile([D, H, R], F8, tag="attnU")
            QT = saout.tile([P, KC, R], BF16, tag="QT")
            KT = saout.tile([P, KC, NC_], BF16, tag="KT")
            V = saout.tile([P, 9, H, D + 1], F8, tag="V")
            wot = saout.tile([D, H, C], F8, tag="wot")

            wbig = ctx2.enter_context(tc.tile_pool(name="wbig", bufs=1))
            with nullcontext():
                # weight stream, consumption order (Pool/SWDGE queue)
                # wv/wk/wq in 2-ko chunks so the bf16 x loads interleave
                # on the DMA engines instead of stalling behind 3.3us blocks
                wvt = wbig.tile([P, KC, C], F8, tag="wvt")
                nc.gpsimd.dma_start(wvt[:, 0:2, :], wv_d[:, 0:2, :])
                identB = consts.tile([P, P], BF16)
                make_identity(nc, identB[:])      # gpsimd memset+affine_select
                identF = consts.tile([P, P], F32)
                make_identity(nc, identF[:])
                nc.gpsimd.dma_start(wvt[:, 2:4, :], wv_d[:, 2:4, :])
                nc.gpsimd.dma_start(wvt[:, 4:6, :], wv_d[:, 4:6, :])
                wkt = wbig.tile([P, KC, C], F8, tag="wkt")
                for j in range(3):
                    nc.gpsimd.dma_start(wkt[:, 2 * j:2 * j + 2, :],
                                        wk_d[:, 2 * j:2 * j + 2, :])
                wqt = wbig.tile([P, KC, C], F8, tag="wqt")
                for j in range(3):
                    nc.gpsimd.dma_start(wqt[:, 2 * j:2 * j + 2, :],
                                        wq_d[:, 2 * j:2 * j + 2, :])
                ckt = wbig.tile([P, KC, C], F8, tag="ckt")
                nc.gpsimd.dma_start(ckt[:], ck_d[:])
                cvt = wbig.tile([P, KC, C], F8, tag="cvt")
                nc.gpsimd.dma_start(cvt[:], cv_d[:])
                nc.gpsimd.dma_start(wot[:], wo_d[:])
                nc.gpsimd.memset(V[:, :, :, D:D + 1], 1.0)
                nc.gpsimd.memset(Vca[:, :, D:D + 1], 1.0)
                # FF weight stream queued right behind the QKV weights (no
                # SBUF aliasing at fp8 sizes, so no anti-deps gate these)
                wff1 = ctx2.enter_context(tc.tile_pool(name="wff1", bufs=12))
                wff2 = ctx2.enter_context(tc.tile_pool(name="wff2", bufs=4))
                pre = ctx2.enter_context(tc.tile_pool(name="pre", bufs=1))
                with tc.tile_pool(name="ps_t0", bufs=3, space="PSUM") as ps_t0, \
                     tc.tile_pool(name="ps_qkv", bufs=3, space="PSUM") as ps_qkv:
                    cT = pre.tile([P, KC, NC_], F8, tag="cT")

                    # warmup transpose (first real one carries a sem wait)
                    ptw = ps_t0.tile([P, P], BF16, tag="tp")
                    nc.tensor.transpose(ptw[:], identB[:], identB[:])

                    def v_chunk(rc, p):
                        for f0, fw, h0, nh in ((0, 512, 0, 8), (512, 256, 8, 4)):
                            pv = ps_qkv.tile([P, 512], F32, tag="pqkv", name="pv")
                            for kp in range(3):
                                nc.tensor.matmul(pv[:p, 0:fw],
                                                 cT[:, 2 * kp:2 * kp + 2,
                                                    rc * P:rc * P + p],
                                                 wvt[:, 2 * kp:2 * kp + 2,
                                                     f0:f0 + fw],
                                                 start=(kp == 0), stop=(kp == 2),
                                                 perf_mode=DR)
                            src = pv[:p, 0:fw].rearrange("p (a b) -> p a b", a=nh)
                            if rc % 3 == 2:
                                nc.scalar.activation(V[:p, rc, h0:h0 + nh, 0:D],
                                                     src, AFT.Copy, scale=WSI)
                            else:
                                nc.vector.tensor_scalar_mul(
                                    V[:p, rc, h0:h0 + nh, 0:D], src, WSI)

                    # LN issued one chunk ahead so the DVE queue never
                    # head-of-line blocks the next chunk's stats behind
                    # this chunk's cT copybacks
                    xn_cur = ln_stats(xf[:, 0, :], P)
                    for rc in range(8):
                        xn_next = (ln_stats(xf[:, rc + 1, :], P) if rc < 7
                                   else ln_stats(face[:], NF))
                        transpose_gb(ps_t0, xn_cur, P, cT, rc * P, 0, 1, rc)
                        v_chunk(rc, P)
                        xn_cur = xn_next
                        if rc == 5 and WARM:
                            # pre-pull the exp table during chunk-phase Act
                            # idle so est0 isn't delayed by the load
                            nc.scalar.activation(actwarm[:, 4:5],
                                                 xf[0:1, 0, 0:1], AFT.Exp)
                    transpose_gb(ps_t0, xn_cur, NF, cT, N, 0, 1)
                    v_chunk(8, NF)

                    # CA K^T and V_ca (text only); copybacks on gpsimd (Act
                    # is about to saturate on exp, DVE on K/Q copybacks)
                    for f in range(KC):
                        pk = ps_qkv.tile([P, 512], F32, tag="pqkv", name="pck")
                        for kp in range(3):
                            nc.tensor.matmul(pk[:, 0:NTP],
                                             ckt[:, 2 * kp:2 * kp + 2,
                                                 bass.ts(f, P)],
                                             ehsT[:, 2 * kp:2 * kp + 2, :],
                                             start=(kp == 0), stop=(kp == 2),
                                             perf_mode=DR)
                        if f % 2 == 0:
                            nc.vector.tensor_scalar_mul(KcaT[:, f, :],
                                                        pk[:, 0:NTP], WSI)
                        else:
                            nc.scalar.activation(KcaT[:, f, :], pk[:, 0:NTP],
                                                 AFT.Copy, scale=WSI)
                    for f0, fw, h0, nh in ((0, 512, 0, 8), (512, 256, 8, 4)):
                        pv = ps_qkv.tile([P, 512], F32, tag="pqkv", name="pcv")
                        for kp in range(3):
                            nc.tensor.matmul(pv[0:NTP, 0:fw],
                                             ehsT[:, 2 * kp:2 * kp + 2, :],
                                             cvt[:, 2 * kp:2 * kp + 2,
                                                 f0:f0 + fw],
                                             start=(kp == 0), stop=(kp == 2),
                                             perf_mode=DR)
                        src = pv[0:NTP, 0:fw].rearrange("p (a b) -> p a b", a=nh)
                        nc.vector.tensor_scalar_mul(Vca[:, h0:h0 + nh, 0:D],
                                                    src, WSI)

            with nullcontext():
                # FF weight stream: issued after the chunk loop so the Pool
                # engine is free for V copybacks during it; lands well before
                # FF1 needs it
                w1cs, w2cs = [], []
                for fc in range(12):
                    if fc % 3 == 0:
                        w2c = wff2.tile([P, KC, C], F8, tag="w2c",
                                        name=f"w2c{fc // 3}")
                        nc.gpsimd.dma_start(
                            w2c[:], w2_d[:, (fc // 3) * KC:(fc // 3 + 1) * KC, :])
                        w2cs.append(w2c)
                    w1c = wff1.tile([P, KC, 2, 256], F8, tag="w1c",
                                    name=f"w1c{fc}")
                    nc.gpsimd.dma_start(w1c[:], w1_d[:, fc, :, :, :])
                    w1cs.append(w1c)

                nc.gpsimd.dma_start(cqt[:], cq_d[:])
                nc.gpsimd.dma_start(cot[:], co_d[:])

                # x1 base = x + tanh(aa)*wo_b, on gpsimd: the obias/xo
                # DMAs land late and these would head-of-line block the DVE
                wobt = tmp1.tile([P, C], F32, tag="wobt")
                nc.gpsimd.tensor_scalar_mul(wobt[:], wobB, tA[:, 0:1])
                for qc in range(2):
                    nc.gpsimd.tensor_add(x1[:, qc, :], xo[:, qc, :], wobt[:])

                if TRUNC >= 2:
                    # ---- self-attention: scores(hp+1) issued before attnV(hp) --
                    NPOOL = os.environ.get("KNPOOL", "") == ""
                    with tc.tile_pool(name="ps_sc", bufs=3, space="PSUM") as ps_sc, \
                         (tc.tile_pool(name="ps_av", bufs=2, space="PSUM")
                          if NPOOL else nullcontext()) as ps_av, \
                         (tc.tile_pool(name="ps_pb", bufs=1, space="PSUM")
                          if NPOOL else nullcontext()) as ps_pb, \
                         (tc.tile_pool(name="ps_po", bufs=2, space="PSUM")
                          if NPOOL else nullcontext()) as ps_po, \
                         tc.tile_pool(name="expp", bufs=10) as expp, \
                         tc.tile_pool(name="expp8", bufs=2) as expp8:
                        ests_all, pavs, pbs, rss = {}, {}, {}, {}

                        def make_qkt(f):
                            # Q^T/K^T column f, streamed just ahead of
                            # sa_scores(f); shares the ps_sc PSUM bufs
                            pq = ps_sc.tile([P, 2, R], F32, tag="psc",
                                            name="pq")
                            for kp in range(3):
                                nc.tensor.matmul(pq[:, 0, :],
                                                 wqt[:, 2 * kp:2 * kp + 2,
                                                     bass.ts(f, P)],
                                                 cT[:, 2 * kp:2 * kp + 2, 0:R],
                                                 start=(kp == 0), stop=(kp == 2),
                                                 perf_mode=DR)
                            nc.vector.tensor_scalar_mul(QT[:, f, :],
                                                        pq[:, 0, :],
                                                        0.125 * WSI)
                            for j0, jw in ((0, 512), (512, 512), (1024, NF)):
                                pk = ps_sc.tile([P, 2, R], F32, tag="psc",
                                                name="pk")
                                pkf = pk[:].rearrange("p a b -> p (a b)")
                                for kp in range(3):
                                    nc.tensor.matmul(pkf[:, 0:jw],
                                                     wkt[:, 2 * kp:2 * kp + 2,
                                                         bass.ts(f, P)],
                                                     cT[:, 2 * kp:2 * kp + 2,
                                                        j0:j0 + jw],
                                                     start=(kp == 0),
                                                     stop=(kp == 2),
                                                     perf_mode=DR)
                                nc.vector.tensor_scalar_mul(
                                    KT[:, f, j0:j0 + jw], pkf[:, 0:jw], WSI)

                        def sa_scores(hp):
                            prs, e8 = [], None
                            for rc in range(9):
                                p = P if rc < 8 else NF
                                psc = ps_sc.tile([P, 2, R], F32, tag="psc")
                                for h01 in range(2):
                                    nc.tensor.matmul(
                                        psc[0:p, h01, :],
                                        KT[h01 * D:(h01 + 1) * D, hp,
                                           rc * P:rc * P + p],
                                        QT[h01 * D:(h01 + 1) * D, hp, :],
                                        start=True, stop=True)
                                if rc < 8:
                                    if rc % 2 == 0:
                                        est = expp.tile([P, 2, 2, R], F8,
                                                        tag="est",
                                                        name=f"est{hp}_{rc//2}")
                                        prs.append(est)
                                    nc.scalar.activation(
                                        est[0:p, :, rc % 2, :],
                                        psc[0:p, :, :], AFT.Exp)
                                else:
                                    e8 = expp8.tile([P, 2, R], F8, tag="est8",
                                                    name=f"est8_{hp}")
                                    nc.scalar.activation(e8[0:p, :, :],
                                                         psc[0:p, :, :], AFT.Exp)
                            ests_all[hp] = (prs, e8)

                        def sa_attnv(hp):
                            # sequential accumulation groups (A then B): two open
                            # groups may not share a 2KB PSUM zero region
                            prs, e8 = ests_all[hp]
                            pav = ps_av.tile([P, 2, R], F32, tag="pav",
                                             name=f"pav{hp}")
                            pavA, pavB = pav[:, 0, :], pav[:, 1, :]
                            for h01 in range(2):
                                dst = pavA if h01 == 0 else pavB
                                for pr in range(4):
                                    for rc01 in range(2):
                                        nc.tensor.matmul(
                                            dst[0:D + 1, :],
                                            V[:, 2 * pr + rc01, 2 * hp + h01, :],
                                            prs[pr][:, h01, rc01, :],
                                            start=(pr == 0 and rc01 == 0),
                                            stop=False)
                                nc.tensor.matmul(dst[0:D + 1, :],
                                                 V[0:NF, 8, 2 * hp + h01, :],
                                                 e8[0:NF, h01, :],
                                                 start=False, stop=True)
                            rs = tmp.tile([1, 2, R], F32R, tag="rs", name=f"rs{hp}")
                            with nc.allow_low_precision(reason="f32r softmax recip"):
                                nc.vector.reciprocal(
                                    rs[:].rearrange("p a b -> p (a b)"),
                                    pav[D:D + 1, :, :].rearrange(
                                        "p a b -> p (a b)"))
                            pavs[hp] = (pavA, pavB)
                            rss[hp] = rs

                        def sa_bcast(hp):
                            pb = ps_pb.tile([D, 2 * R], F32, tag="pb", name=f"pb{hp}")
                            nc.tensor.matmul(pb[:], ones_r[0:1, 0:D],
                                             rss[hp][:].rearrange("p a b -> p (a b)"),
                                             start=True, stop=True)
                            # DVE may read only one PSUM operand per instruction:
                            # stage the broadcast reciprocals in SBUF (Act for the
                            # epilogue head-pairs where the exp stream has drained)
                            # x WS so attnU lands in fp8's normal range; the
                            # extra WS is cancelled in tA2 at the O-proj
                            pbs_sb = tmp.tile([D, 2 * R], BF16, tag="pbs",
                                              name=f"pbs{hp}")
                            if hp >= 4:
                                nc.scalar.activation(pbs_sb[:], pb[:], AFT.Copy,
                                                     scale=WS)
                            else:
                                nc.vector.tensor_scalar_mul(pbs_sb[:], pb[:], WS)
                            pbs[hp] = pbs_sb

                        def sa_divide(hp):
                            pavA, pavB = pavs[hp]
                            pb = pbs[hp]
                            nc.vector.tensor_mul(attnU[0:D, 2 * hp, :], pavA[0:D, :],
                                                 pb[:, 0:R])
                            nc.vector.tensor_mul(attnU[0:D, 2 * hp + 1, :],
                                                 pavB[0:D, :], pb[:, R:2 * R])

                        poq = ([ps_po.tile([P, 384], F32, tag="poq",
                                           name=f"poq{j}") for j in range(2)]
                               if NPOOL else None)

                        def sa_oproj_step(hp):
                            # qc0 half of the O-proj, folded into the attention
                            # loop; fp8 DoubleRow pairs the two heads of hp
                            for j in range(2):
                                nc.tensor.matmul(
                                    poq[j][:],
                                    attnU[0:D, 2 * hp:2 * hp + 2, 0:P],
                                    wot[:, 2 * hp:2 * hp + 2,
                                        384 * j:384 * (j + 1)],
                                    start=(hp == 0), stop=(hp == HP - 1),
                                    perf_mode=DR)

                        KSA = int(os.environ.get("KSA", "9"))
                        if KSA < 9:
                            make_qkt(0)
                            sa_scores(0)
                            if KSA >= 2:
                                sa_attnv(0)
                            if KSA >= 3:
                                sa_bcast(0)
                            if KSA >= 4:
                                sa_divide(0)
                        if KSA >= 9:
                         make_qkt(0)
                         sa_scores(0)
                         make_qkt(1)
                         sa_scores(1)
                         make_qkt(2)
                         sa_attnv(0)
                         for hp in range(2, HP):
                            sa_scores(hp)
                            sa_bcast(hp - 2)
                            sa_divide(hp - 2)
                            sa_attnv(hp - 1)
                            sa_oproj_step(hp - 2)
                            if hp + 1 < HP:
                                make_qkt(hp + 1)
                         sa_bcast(HP - 2)
                         sa_divide(HP - 2)
                         sa_attnv(HP - 1)
                         sa_oproj_step(HP - 2)
                         sa_bcast(HP - 1)
                         sa_divide(HP - 1)
                         sa_oproj_step(HP - 1)
                        if WARM and KSA >= 9:
                            nc.scalar.activation(actwarm[:, 1:2],
                                                 attnU[0:1, H - 1, 0:1], AFT.Sqrt,
                                                 scale=0.0, bias=eps_t[0:1, 0:1])
                        # x1 qc0 while still inside the attention pools
                        for j in range(2 if KSA >= 9 else 0):
                            nc.vector.scalar_tensor_tensor(
                                x1[:, 0, 384 * j:384 * (j + 1)], poq[j][:],
                                tA2[:, 0:1], x1[:, 0, 384 * j:384 * (j + 1)],
                                mybir.AluOpType.mult, mybir.AluOpType.add)

                if TRUNC >= 3:
                    # ---------------- FF (entry overlapped with O-proj qc1) ---
                    with tc.tile_pool(name="ffp", bufs=1) as ffp, \
                         tc.tile_pool(name="ps_tf", bufs=2, space="PSUM") as ps_tf:
                        hT = ffp.tile([P, KC, R], F8, tag="hT")

                        def ff_norm(rc):
                            # normalized hT input for row-chunk rc
                            if fast_ln2:
                                # ln2_g == 1, ln2_b == 0: LN(LN(x)) == LN(x) up
                                # to O(eps) ~ 5e-6 -- skip the second stats pass
                                return ln_stats(x1[:, rc, :], P)
                            xn = ln_stats(x1[:, rc, :], P)
                            y = tmp1.tile([P, C], BF16, tag="ffy")
                            nc.vector.tensor_mul(y[:], xn[:], g2b[:, 0, :])
                            nc.vector.tensor_add(y[:], y[:], g2b[:, 1, :])
                            return ln_stats(y[:], P)

                        if not fast_ln2:
                            g2b = ffp.tile([P, 2, C], BF16, tag="g2b")
                            nc.gpsimd.dma_start(g2b[:], bcast_d[:, 0:2, :])
                        # rc0 LN issued first: its Act/DVE work runs while the
                        # PE does the qc1 O-proj below
                        xn0 = ff_norm(0)

                        # ---- O-proj qc1 + gated residual -> x1 ----
                        with tc.tile_pool(name="ps_pr", bufs=2,
                                          space="PSUM") as ps_pr:
                            for f0, fw in (((0, 384), (384, 384))
                                           if KSA >= 9 else ()):
                                po = ps_pr.tile([P, 384], F32, tag="po")
                                for hp in range(HP):
                                    nc.tensor.matmul(
                                        po[:],
                                        attnU[0:D, 2 * hp:2 * hp + 2, P:2 * P],
                                        wot[:, 2 * hp:2 * hp + 2, f0:f0 + fw],
                                        start=(hp == 0), stop=(hp == HP - 1),
                                        perf_mode=DR)
                                nc.vector.scalar_tensor_tensor(
                                    x1[:, 1, f0:f0 + fw], po[:], tA2[:, 0:1],
                                    x1[:, 1, f0:f0 + fw],
                                    mybir.AluOpType.mult, mybir.AluOpType.add)

                        if fast_ln2:
                            xn1 = ff_norm(1)
                            transpose_gb(ps_tf, xn0, P, hT, 0, 2, 3, 0,
                                         pool_mix=True)
                        else:   # double-LN recycles the xnb bufs: rc0 first
                            transpose_gb(ps_tf, xn0, P, hT, 0, 2, 3, 0,
                                         pool_mix=True)
                            xn1 = ff_norm(1)
                        transpose_gb(ps_tf, xn1, P, hT, P, 2, 3, 1,
                                     pool_mix=True)

                        if WARM:
                            nc.scalar.activation(actwarm[:, 2:3],
                                                 hT[0:1, KC - 1, R - 1:R], AFT.Gelu)
                        actT = ffp.tile([P, 24, R], F8, tag="actT")
                        ffTb = ffp.tile([P, KC, R], BF16, tag="ffTb")
                        with tc.tile_pool(name="ps_h1", bufs=3,
                                          space="PSUM") as ps_h1:
                            for fc in range(12):
                                w1c = w1cs[fc]
                                # 4 chains in one tile; groups per 2KB zero
                                # region run sequentially (ag inner)
                                pag = ps_h1.tile([P, 2, 2, R], F32, tag="ph1",
                                                 name="pag")
                                for fi in range(2):
                                    for ag in range(2):
                                        for kp in range(3):
                                            nc.tensor.matmul(
                                                pag[:, fi, ag, :],
                                                w1c[:, 2 * kp:2 * kp + 2, ag,
                                                    bass.ts(fi, P)],
                                                hT[:, 2 * kp:2 * kp + 2, :],
                                                start=(kp == 0), stop=(kp == 2),
                                                perf_mode=DR)
                                gl = tmp.tile([P, 2, R], F32, tag="gl")
                                nc.scalar.activation(gl[:], pag[:, :, 1, :],
                                                     AFT.Gelu, scale=1.0 / W1S)
                                nc.vector.tensor_mul(actT[:, 2 * fc:2 * fc + 2, :],
                                                     pag[:, :, 0, :], gl[:])

                        # FF2: f-outer so each f's 12-matmul chain completes
                        # before the next (no two open groups per PSUM bank)
                        with tc.tile_pool(name="ps_f2", bufs=3,
                                          space="PSUM") as ps_f2:
                            pf2 = [ps_f2.tile([P, 2, R], F32, tag="pf",
                                              name=f"pf{j}") for j in range(3)]
                            pfs = [pf2[f // 2][:, f % 2, :] for f in range(KC)]
                            for f in range(KC):
                                for qb in range(4):
                                    for kp in range(3):
                                        nc.tensor.matmul(
                                            pfs[f][:],
                                            w2cs[qb][:, 2 * kp:2 * kp + 2,
                                                     bass.ts(f, P)],
                                            actT[:, qb * KC + 2 * kp:
                                                 qb * KC + 2 * kp + 2, :],
                                            start=(qb == 0 and kp == 0),
                                            stop=(qb == 3 and kp == 2),
                                            perf_mode=DR)
                                # tanh(ad)/(W1S*W2S) folded in; bf16 out for
                                # cheap transposes
                                nc.scalar.activation(ffTb[:, f, :], pfs[f][:],
                                                     AFT.Copy, scale=tD2[:, 0:1])
                                # x2 and x2^T produced per-f right behind the
                                # FF2 chain so CA entry has no transpose tail
                                for qc in range(2):
                                    pt = ps_tf.tile([P, P], BF16, tag="tp")
                                    nc.tensor.transpose(
                                        pt[:], ffTb[:, f, bass.ts(qc, P)],
                                        identB[:])
                                    nc.vector.tensor_add(
                                        x2[:, qc, bass.ts(f, P)], pt[:],
                                        x1[:, qc, bass.ts(f, P)])
                                    ptc = ps_f2.tile([P, P], F32, tag="tpc")
                                    nc.tensor.transpose(
                                        ptc[:], x2[:, qc, bass.ts(f, P)],
                                        identF[:])
                                    if qc == 0:
                                        nc.vector.tensor_copy(
                                            x2T[:, f, bass.ts(qc, P)], ptc[:])
                                    else:
                                        nc.scalar.activation(
                                            x2T[:, f, bass.ts(qc, P)], ptc[:],
                                            AFT.Identity)

        ctx2.close()
        if TRUNC >= 4:
            # ---------------- cross-attention (shift-free) ----------------
            with tc.tile_pool(name="cap", bufs=1) as cap:
                if WARM:
                    nc.scalar.activation(actwarm[:, 3:4], x2[0:1, 1, C - 1:C],
                                         AFT.Exp)
                x2c = cap.tile([P, 2, C], F32, tag="x2c")
                for qc in range(2):
                    nc.gpsimd.tensor_add(x2c[:, qc, :], x2[:, qc, :], cobB[:])
                qcaT = cap.tile([P, KC, R], BF16, tag="qcaT")
                with nullcontext():
                    with tc.tile_pool(name="ps_ca", bufs=3, space="PSUM") as ps_ca:
                        for f in range(KC):
                            pq = ps_ca.tile([P, R], F32, tag="pca", name="pcq")
                            for kp in range(3):
                                nc.tensor.matmul(pq[:],
                                                 cqt[:, 2 * kp:2 * kp + 2,
                                                     bass.ts(f, P)],
                                                 x2T[:, 2 * kp:2 * kp + 2, :],
                                                 start=(kp == 0), stop=(kp == 2),
                                                 perf_mode=DR)
                            if f % 2 == 0:
                                nc.vector.tensor_scalar_mul(qcaT[:, f, :], pq[:],
                                                            0.125 * WSI)
                            else:
                                nc.scalar.activation(qcaT[:, f, :], pq[:],
                                                     AFT.Copy,
                                                     scale=0.125 * WSI)

                    attnCT = cap.tile([P, HP, R], F8, tag="attnCT")
                    outt = cap.tile([P, 2, C], F32, tag="outt")
                    with tc.tile_pool(name="ps_cs", bufs=2, space="PSUM") as ps_cs, \
                         tc.tile_pool(name="ps_cav", bufs=2, space="PSUM") as ps_cav, \
                         tc.tile_pool(name="ps_crs", bufs=2, space="PSUM") as ps_crs, \
                         tc.tile_pool(name="ps_cpb", bufs=2, space="PSUM") as ps_cpb, \
                         tc.tile_pool(name="expc", bufs=4) as expc:
                        cests, cpavs, cpbs, crss = {}, {}, {}, {}

                        def ca_scores(hp):
                            estc = expc.tile([NTP, 2, R], F8, tag="estc",
                                             name=f"estc{hp}")
                            nc.gpsimd.memset(estc[:, :, :], 0.0)
                            psc = ps_cs.tile([P, 2, R], F32, tag="pcs")
                            for h01 in range(2):
                                nc.tensor.matmul(psc[0:NTP, h01, :],
                                                 KcaT[h01 * D:(h01 + 1) * D, hp, :],
                                                 qcaT[h01 * D:(h01 + 1) * D, hp, :],
                                                 start=True, stop=True)
                            nc.scalar.activation(estc[0:NT, :, :], psc[0:NT, :, :],
                                                 AFT.Exp)
                            cests[hp] = estc

                        def ca_attnv(hp):
                            estc = cests[hp]
                            # h0 -> partitions 0:64, h1 -> 64:128 (no shift DMA);
                            # row-sums via the Vca ones-column over both heads
                            pav = ps_cav.tile([P, R], F32, tag="pcav",
                                              name=f"cpav{hp}")
                            nc.tensor.matmul(pav[0:D, :], Vca[:, 2 * hp, 0:D],
                                             estc[:, 0, :], start=True, stop=True)
                            nc.tensor.matmul(pav[D:P, :], Vca[:, 2 * hp + 1, 0:D],
                                             estc[:, 1, :], start=True, stop=True)
                            prs = ps_crs.tile([1, 2, R], F32, tag="crsum",
                                              name=f"crsum{hp}")
                            nc.tensor.matmul(
                                prs[:].rearrange("p a b -> p (a b)"),
                                Vca[:, 0, D:D + 1],
                                estc[:, :, :].rearrange("p a b -> p (a b)"),
                                start=True, stop=True)
                            rs = tmp.tile([1, 2, R], F32R, tag="crs",
                                          name=f"crs{hp}")
                            with nc.allow_low_precision(reason="f32r softmax recip"):
                                nc.vector.reciprocal(
                                    rs[:].rearrange("p a b -> p (a b)"),
                                    prs[:].rearrange("p a b -> p (a b)"))
                            cpavs[hp] = pav
                            crss[hp] = rs

                        def ca_bcast(hp):
                            pb = ps_cpb.tile([P, 2 * R], F32, tag="cpb",
                                             name=f"cpb{hp}")
                            nc.tensor.matmul(pb[:], ones_r[0:1, :],
                                             crss[hp][:].rearrange("p a b -> p (a b)"),
                                             start=True, stop=True)
                            # x WS so attnCT lands in fp8 normal range; the
                            # extra WS cancels against cot's host-side x WS
                            pbs_sb = tmp.tile([P, 2 * R], BF16, tag="cpbs",
                                              name=f"cpbs{hp}")
                            nc.scalar.activation(pbs_sb[:], pb[:], AFT.Copy,
                                                 scale=WS)
                            cpbs[hp] = pbs_sb

                        def ca_divide(hp):
                            pav, pb = cpavs[hp], cpbs[hp]
                            nc.vector.tensor_mul(attnCT[0:D, hp, :], pav[0:D, :],
                                                 pb[0:D, 0:R])
                            nc.vector.tensor_mul(attnCT[D:P, hp, :], pav[D:P, :],
                                                 pb[D:P, R:2 * R])

                        ca_scores(0)
                        ca_scores(1)
                        ca_attnv(0)
                        for hp in range(2, HP):
                            ca_scores(hp)
                            ca_bcast(hp - 2)
                            ca_attnv(hp - 1)
                            ca_divide(hp - 2)
                        ca_bcast(HP - 2)
                        ca_attnv(HP - 1)
                        ca_divide(HP - 2)
                        ca_bcast(HP - 1)
                        ca_divide(HP - 1)

                    # CA O-proj + bias + residual -> out; hq-outer so the
                    # first-pair matmuls of all 4 blocks run before the last
                    # attnCT divides land
                    with tc.tile_pool(name="ps_co", bufs=4, space="PSUM") as ps_co:
                        blocks = [(qc, f0, fw) for qc in range(2)
                                  for f0, fw in ((0, 512), (512, 256))]
                        pos = [ps_co.tile([P, 512], F32, tag="pco",
                                          name=f"pco{i}") for i in range(4)]
                        for hq in range(3):
                            for i, (qc, f0, fw) in enumerate(blocks):
                                nc.tensor.matmul(pos[i][:, 0:fw],
                                                 attnCT[:, 2 * hq:2 * hq + 2,
                                                        bass.ts(qc, P)],
                                                 cot[:, 2 * hq:2 * hq + 2,
                                                     f0:f0 + fw],
                                                 start=(hq == 0),
                                                 stop=(hq == 2),
                                                 perf_mode=DR)
                        for i, (qc, f0, fw) in enumerate(blocks):
                            nc.vector.scalar_tensor_tensor(
                                outt[:, qc, f0:f0 + fw], pos[i][:, 0:fw],
                                1.0 / (WS * WS), x2c[:, qc, f0:f0 + fw],
                                mybir.AluOpType.mult, mybir.AluOpType.add)
                            nc.sync.dma_start(
                                out_d[qc * P:(qc + 1) * P, f0:f0 + fw],
                                outt[:, qc, f0:f0 + fw])

        if TRUNC < 4:
            dbg_out(xo[:, 0, :])
    nc.compile()
    return nc


def _pack_inputs(inputs):
    """Host-side packing: bf16 weight blobs in SBUF layout + per-core x."""
    import ml_dtypes
    bf16 = ml_dtypes.bfloat16
    fp8 = ml_dtypes.float8_e4m3
    f32 = lambda a: np.ascontiguousarray(np.asarray(a), dtype=np.float32)

    def kof(w, dt=bf16, scale=1.0):  # [768, F] -> [128, 6, F]  ((ko p) f -> p ko f)
        w = f32(w) * np.float32(scale)
        return np.ascontiguousarray(
            w.reshape(KC, P, w.shape[1]).transpose(1, 0, 2).astype(dt))

    common = {
        "wv": kof(inputs["sa_wv"], fp8, WS),
        "wk": kof(inputs["sa_wk"], fp8, WS),
        "wq": kof(inputs["sa_wq"], fp8, WS),
        "ck": kof(inputs["ca_wk"], fp8, WS),
        "cv": kof(inputs["ca_wv"], fp8, WS),
        "wo": np.ascontiguousarray(
            (np.asarray(inputs["sa_wo"], np.float32) * np.float32(WS))
            .reshape(H, D, C).transpose(1, 0, 2).astype(fp8)),
        "cq": kof(inputs["ca_wq"], fp8, WS),
        "co": kof(inputs["ca_wo"], fp8, WS),
    }
    # w1 [768, 6144] -> [p, fc(12), ko(6), ag(2), 256], fp8 scaled by W1S
    w1 = f32(inputs["ff_w1"]).reshape(KC, P, 2, 12, 256) * np.float32(16.0)
    common["w1"] = np.ascontiguousarray(w1.transpose(1, 3, 0, 2, 4).astype(fp8))
    # w2 [3072, 768] -> [p, kq(24), 768], fp8 scaled by W2S
    w2 = f32(inputs["ff_w2"]).reshape(24, P, C) * np.float32(64.0)
    common["w2"] = np.ascontiguousarray(w2.transpose(1, 0, 2).astype(fp8))
    # packed LN vectors (transposed form): {ln1_g, ln1_b, ff_ln_g, ff_ln_b}
    lnvT = np.stack([f32(inputs[k]) for k in
                     ("ln1_g", "ln1_b", "ff_ln_g", "ff_ln_b")], axis=-1)
    common["lnvT"] = np.ascontiguousarray(lnvT.reshape(KC, P, 4).transpose(1, 0, 2))
    # broadcast vectors: {ln2_g, ln2_b, sa_wo_b, ca_wo_b}
    bc = np.stack([f32(inputs[k]) for k in
                   ("ln2_g", "ln2_b", "sa_wo_b", "ca_wo_b")], axis=0)
    common["bcast"] = np.ascontiguousarray(np.broadcast_to(bc[None], (P, 4, C)))
    ta = np.tanh(np.float32(inputs["alpha_attn"]))
    td = np.tanh(np.float32(inputs["alpha_dense"]))
    common["alph"] = np.array([[ta, td / np.float32(W1S * W2S),
                                ta / np.float32(WS * WS)]], np.float32)

    hs = f32(inputs["hidden_states"])
    ehs = f32(inputs["encoder_hidden_states"])
    in_maps = []
    for c in range(8):
        b, r = c // 4, c % 4
        m = dict(common)
        # own rows first, then the rest of the batch (order-invariant attn)
        perm = np.r_[r * R:(r + 1) * R, 0:r * R, (r + 1) * R:N]
        xp = hs[b][perm]
        m["x_full"] = np.ascontiguousarray(xp)
        m["xb"] = np.ascontiguousarray(
            xp.reshape(8, P, C).transpose(1, 0, 2).astype(bf16))
        m["face"] = np.ascontiguousarray(ehs[b, NT:L])
        tT = np.zeros((C, NTP), np.float32)
        tT[:, :NT] = ehs[b, :NT].T
        m["ehsT"] = np.ascontiguousarray(
            tT.reshape(KC, P, NTP).transpose(1, 0, 2).astype(fp8))
        in_maps.append(m)
    return in_maps


def _run_coresim(nc, in_maps):
    """Host-side execution fallback (bass interpreter); the python interp
    lacks Gelu, so patch it (scoped) with the exact erf-based gelu."""
    import concourse.bass_interp as bi
    from concourse.bass_interp import CoreSim
    try:
        from scipy.special import erf
    except ImportError:
        def erf(x):
            import math
            v = np.vectorize(math.erf)
            return v(x)
    orig = bi.InstructionExecutor.visit_InstActivation

    def patched(self, instruction, reg_snapshot=None):
        if instruction.func == mybir.ActivationFunctionType.Gelu:
            instruction.func = mybir.ActivationFunctionType.Identity
            try:
                r = orig(self, instruction, reg_snapshot=reg_snapshot)
            finally:
                instruction.func = mybir.ActivationFunctionType.Gelu
            out_ap = instruction.outs[0]
            ov = self.view_ap(out_ap, bi.Direction.WRITE, instruction,
                              reg_snapshot=reg_snapshot)
            x = np.asarray(ov[:], np.float64)
            ov[:] = (x * 0.5 * (1 + erf(x / np.sqrt(2)))).astype(ov.dtype)
            return r
        return orig(self, instruction, reg_snapshot=reg_snapshot)

    bi.InstructionExecutor.visit_InstActivation = patched
    try:
        outs = []
        for m in in_maps:
            sim = CoreSim(nc, require_finite=False, require_nnan=False)
            for name, arr in m.items():
                sim.tensor(name)[:] = arr
            sim.simulate()
            outs.append(np.array(sim.tensor("out_own")))
    finally:
        bi.InstructionExecutor.visit_InstActivation = orig
    return outs


def kernel(**inputs):
    fast_ln2 = bool(np.all(np.asarray(inputs["ln2_g"]) == 1.0)
                    and np.all(np.asarray(inputs["ln2_b"]) == 0.0))
    key = ("nc", fast_ln2)
    if key not in _cache:
        _cache[key] = build(fast_ln2)
    nc = _cache["nc"] = _cache[key]

    in_maps = _pack_inputs(inputs)
    try:
        res = run_bass_kernel_spmd(nc, in_maps, core_ids=list(range(8)))
        core_outs = [res.results[c]["out_own"] for c in range(8)]
    except Exception:
        # PJRT/axon path unavailable or rejecting the NEFF: execute on the
        # host bass interpreter instead (bit-faithful per-instruction).
        core_outs = _run_coresim(nc, in_maps)
    out = np.empty((B, N, C), np.float32)
    for c in range(8):
        b, r = c // 4, c % 4
        out[b, r * R:(r + 1) * R] = core_outs[c]
    return out

